# revision 15
# baseline (speedup 1.0000x reference)
"""Distributed attention kernel for Trainium2 (8 NeuronCores, SPMD).

Problem: B=16 batches of single-query attention over NK=4096 keys,
EMBED=1024, ATTN=256, shared kq projection + v projection.

Math restructuring (exact up to float reassociation):
  - scores = k @ qt + const   where qt = W_kq @ (W_kq^T q + b_kq) / 16
    (constant offset -> softmax invariant, dropped)
  - out = (attn @ v) @ W_v + b_v   (attn sums to 1)

V2 design (per core: 2 batches, 4 chunks of 1024 tokens each):
  - batch 0 scores on DVE: k int8 (x32), scalar_tensor_tensor with
    qtb broadcast (qt scale 1/(16*32)), accum -> smat cols. 8 stt/chunk.
  - batch 1 scores on PE: k^T stored fp8-e3m4 [e-slice part, tok free];
    row-form matmuls lhsT=qt col slice [128x1] bf16 (scale 1/16),
    rhs=kT moving -> PSUM [1x512] rows; ACT exp rows (accum_out = Z
    partials); PE transposes [1x128]->[128x1] -> pmat cols.
  - v fp8-e3m4 for BOTH batches' attn@v (w) matmuls (lhsT pmat bf16
    x rhs fp8 moving validated on HW, rel err ~1e-7 vs exact).
  - DMA/core ~18.4MB (vs 30.5 baseline): ka 4MB int8 + kt 4MB e3m4 +
    v 8MB e3m4 + W_v 2MB bf16 + small. One HWDGE queue ~400-430 GB/s.
  - W_v on the gpsimd/SWDGE queue (otherwise idle).

Synchronization rules (enforced by CoreSim's race detector):
  - DMA-completion sems: one per BUFFER SLOT (16 SDMA engines increment
    independently; prefix waits on a shared counter are unsound).
  - Same-engine RAW needs a SELF-WAIT on the engine's own sem (DVE
    reads of qtb0 after QTBSB0; DVE INVZ1 after ZRED1).
  - PSUM bank rule: concurrent PE-write + DVE/ACT-read of one bank is
    fatal; every bank handoff is serialized via the event chain.

PSUM bank map:
  banks 0-1: warmup -> qt1 -> w0 accumulator
  banks 2-3: qt0 -> w1 accumulator
  banks 4-5: qprow1 [0:1,0:256] (b4) -> qtb0 broadcast -> score rows
             [1x512] x2 (b4=half0, b5=half1) -> proj out [2x1024]
  bank 6   : qtcol transposes -> per-chunk prob transposes (bf16)
  bank 7   : qprow0 [0:1,256:512], qp folds [:,0:4], w folds [:,32:48],
             Z0 [0:1,300:301]

Raw bass (not Tile): walrus rejects >1 embedded sync-wait per compute
instruction; standalone sequencer waits with python-precomputed
semaphore tick tables (ticks = position in each engine's event list;
the lists must match engine program order exactly).
"""

import contextlib

import numpy as np

try:
    import concourse.bass as bass  # noqa: F401
except ImportError:
    import sys

    sys.path.insert(0, "/opt/trn_rl_repo")

B = 16
NCORES = 8
BPC = B // NCORES
NK = 4096
E = 1024
A = 256
CHUNK = 8                      # k-tiles (128 tok) per chunk
NCHUNK = NK // (128 * CHUNK)   # 4 chunks per batch
NSUB = NK // 128               # 32 tiles per batch
SUB = 2                        # b0 exp granularity in tiles
KABUFS = 3
KTBUFS = 4
VBUFS = 5
QT_SCALE0 = 1.0 / (16.0 * 32.0)   # b0: int8 k carries x32
QT_SCALE1 = 1.0 / 16.0            # b1: e3m4 k carries real values

# period order: b1 (PE-scored) first so the PE has work at startup;
# all b0 (DVE) chunks early so the stt pipeline drains before the tail;
# the last two periods are b1 -> short PE-only tail chain
PERIODS = [(1, 0), (0, 0), (0, 1), (1, 1), (0, 2), (0, 3), (1, 2), (1, 3)]


def _build_nc():
    import concourse.bass as bass
    from concourse import mybir

    FP = mybir.dt.float32
    BF = mybir.dt.bfloat16
    I8 = mybir.dt.int8
    E3 = mybir.dt.float8e3
    F16 = mybir.dt.float16
    AL = mybir.AluOpType
    AF = mybir.ActivationFunctionType

    nc = bass.Bass()
    q_d = nc.declare_dram_parameter("q", [128, 8, BPC], F16, isOutput=False)
    ka_d = nc.declare_dram_parameter("ka", [NCHUNK, 128, CHUNK * E], I8,
                                     isOutput=False)
    kt_d = nc.declare_dram_parameter("kt", [NCHUNK, 128, CHUNK * E], F16,
                                     isOutput=False)
    v_d = nc.declare_dram_parameter("v", [BPC, NCHUNK, 128, CHUNK * E], E3,
                                    isOutput=False)
    wkq_d = nc.declare_dram_parameter("W_kq", [E, A], F16, isOutput=False)
    wkqT_d = nc.declare_dram_parameter("W_kqT", [A, E], F16, isOutput=False)
    bkq_d = nc.declare_dram_parameter("b_kq", [A], FP, isOutput=False)
    wv_d = nc.declare_dram_parameter("W_v", [E, E], F16, isOutput=False)
    bv_d = nc.declare_dram_parameter("b_v", [BPC, E], FP, isOutput=False)
    out_d = nc.declare_dram_parameter("out", [BPC, E], FP, isOutput=True)

    with contextlib.ExitStack() as st:
        def sb(name, shape, dt=FP):
            return st.enter_context(nc.sbuf_tensor(name, shape, dt))

        # ---- SBUF ----
        wkq_sb = sb("wkq_sb", [128, 8, A], F16)
        wkqT_sb = sb("wkqT_sb", [128, 2, E], F16)
        wv_sb = sb("wv_sb", [128, 8, E], F16)
        q_col = sb("q_col", [128, 8, BPC], F16)
        bkq_row = sb("bkq_row", [1, A])
        bv_row = sb("bv_row", [BPC, E])
        ones_col = sb("ones_col", [128, 1])      # fp32 (Z0 rhs)
        ones_bf = sb("ones_bf", [1, 128], F16)    # bf16 (fold/bcast/identity)
        ka = [sb(f"ka{i}", [128, CHUNK, E], I8) for i in range(KABUFS)]
        kt = [sb(f"kt{i}", [128, CHUNK, E], F16) for i in range(KTBUFS)]
        vt = [sb(f"vt{i}", [128, CHUNK, E], E3) for i in range(VBUFS)]
        junks = [sb(f"junk{i}", [128, E], BF) for i in range(CHUNK)]
        qpr_sb = [sb(f"qpr_sb{b}", [1, A], F16) for b in range(BPC)]
        qp_sb = sb("qp_sb", [128, 4], F16)        # cols (ac,b): ac*2+b
        qt_sb = [sb(f"qt_sb{b}", [1, E], F16) for b in range(BPC)]
        qtb0_sb = sb("qtb0_sb", [128, E], F16)    # b0 broadcast
        qtc_sb = sb("qtc_sb", [128, 16], F16)     # b1 qt cols (stride 2)
        smat0 = sb("smat0", [128, NSUB])
        pmat0 = sb("pmat0", [128, NSUB], F16)
        pmat1 = sb("pmat1", [128, 2 * NSUB], F16)  # tile t at col 2t
        p1row = [sb(f"p1row{i}", [1, E], F16) for i in range(2)]
        zparts = sb("zparts", [1, 2 * NCHUNK])   # b1 Z partials (c,half)
        zsum1 = sb("zsum1", [1, 1])
        zredc0 = sb("zredc0", [128, 1])
        invz = [sb(f"invz{b}", [1, 1]) for b in range(BPC)]
        w_row = [sb(f"w_row{b}", [1, E], F16) for b in range(BPC)]
        wcb = sb("wcb", [128, 16], F16)           # fold cols (dc,b): dc*2+b
        o_sb = sb("o_sb", [BPC, E])

        # ---- PSUM ----
        ps_w0 = st.enter_context(nc.psum_tensor([128, 1024], FP))  # 0-1
        ps_w1 = st.enter_context(nc.psum_tensor([128, 1024], FP))  # 2-3
        ps_s = st.enter_context(nc.psum_tensor([128, 1024], FP))   # 4-5
        ps_t = st.enter_context(nc.psum_tensor([128, 512], F16))    # 6
        ps_m = st.enter_context(nc.psum_tensor([128, 512], FP))    # 7

        # ---- semaphores ----
        sW1 = st.enter_context(nc.semaphore("sW1"))  # q+wkq -> 32
        sW2 = st.enter_context(nc.semaphore("sW2"))  # bkq+wkqT -> 32
        sWV = st.enter_context(nc.semaphore("sWV"))
        sBV = st.enter_context(nc.semaphore("sBV"))
        sKA = [st.enter_context(nc.semaphore(f"sKA{i}")) for i in range(KABUFS)]
        sKT = [st.enter_context(nc.semaphore(f"sKT{i}")) for i in range(KTBUFS)]
        sV = [st.enter_context(nc.semaphore(f"sV{i}")) for i in range(VBUFS)]
        sOUT = st.enter_context(nc.semaphore("sOUT"))
        sPE = st.enter_context(nc.semaphore("sPE"))
        sDVE = st.enter_context(nc.semaphore("sDVE"))
        sACT = st.enter_context(nc.semaphore("sACT"))

        blk = st.enter_context(nc.Block())

        def ticks(seq):
            assert len(set(seq)) == len(seq), "dup event"
            return {ev: i + 1 for i, ev in enumerate(seq)}

        # ---------- event sequences (must match program order) ----------
        pe_seq = ["QPROW0", "QPROW1", "QPF0", "QPF1", "QT0", "QT1",
                  "QTB0", "QTC1"]
        # periods p0..p4 in order; hand-scheduled tail for p5..p7 so the
        # late-arriving data (v of the last periods) doesn't block earlier
        # score work in PE program order
        for pi, (b, c) in enumerate(PERIODS[:5]):
            if b == 0:
                for h in range(CHUNK // SUB):
                    pe_seq.append(f"W0_{c}_{h}")
            else:
                pe_seq.append(f"S1_{c}_0")
                pe_seq.append(f"S1_{c}_1")
                pe_seq.append(f"T1_{c}_0")
                pe_seq.append(f"T1_{c}_1")
                for h in range(CHUNK // SUB):
                    pe_seq.append(f"W1_{c}_{h}")
        pe_seq += ["S1_2_0", "S1_2_1", "T1_2_0", "T1_2_1",
                   "S1_3_0", "S1_3_1", "T1_3_0", "T1_3_1"]
        for h in range(CHUNK // SUB):
            pe_seq.append(f"W1_2_{h}")
        for h in range(CHUNK // SUB):
            pe_seq.append(f"W0_3_{h}")
        pe_seq.append("Z0")
        for h in range(CHUNK // SUB):
            pe_seq.append(f"W1_3_{h}")
        pe_seq += ["FOLD0", "FOLD1", "PROJ"]
        PE = ticks(pe_seq)

        dve_seq = ["MS1", "MS2", "QPRSB0", "QPRSB1", "QPSB", "QTBSB0",
                   "QTCSB1"]
        for b, c in PERIODS:
            if b == 0:
                for j in range(CHUNK):
                    dve_seq.append(f"STT0_{c}_{j}")
                if c == NCHUNK - 1:
                    dve_seq += ["ZRED0", "INVZ0"]
        dve_seq += ["ZRED1", "INVZ1", "WCOL", "PROJCP"]
        DVE = ticks(dve_seq)

        act_seq = ["QTSB0", "QTSB1"]
        for b, c in PERIODS:
            if b == 0:
                for h in range(CHUNK // SUB):
                    act_seq.append(f"EXP0_{c}_{h}")
            else:
                act_seq.append(f"EXPR1_{c}_0")
                act_seq.append(f"EXPR1_{c}_1")
                act_seq.append(f"PCOL1_{c}")
        act_seq += ["WROW0", "WROW1"]
        ACT = ticks(act_seq)

        # ---------- SYNC: HWDGE DMAs ----------
        @blk.sync
        def _(sync):
            sync.dma_start(out=q_col[:], in_=q_d[:]).then_inc(sW1, 16)
            sync.dma_start(
                out=wkq_sb[:], in_=wkq_d[:].rearrange("(dc p) a -> p dc a", p=128)
            ).then_inc(sW1, 16)
            sync.dma_start(out=kt[0][:], in_=kt_d[:][0]).then_inc(sKT[0], 16)
            sync.dma_start(out=ka[0][:], in_=ka_d[:][0]).then_inc(sKA[0], 16)
            sync.dma_start(out=bkq_row[:], in_=bkq_d[:][None, :]).then_inc(sW2, 16)
            sync.dma_start(
                out=wkqT_sb[:], in_=wkqT_d[:].rearrange("(ac p) d -> p ac d", p=128)
            ).then_inc(sW2, 16)
            sync.dma_start(out=bv_row[:], in_=bv_d[:]).then_inc(sBV, 16)

            def vdma(vpi):
                vb, vc = PERIODS[vpi]
                if vpi >= VBUFS:
                    bp, cp = PERIODS[vpi - VBUFS]
                    sync.wait_ge(sPE, PE[f"W{bp}_{cp}_{CHUNK // SUB - 1}"])
                sync.dma_start(out=vt[vpi % VBUFS][:],
                               in_=v_d[:][vb, vc]).then_inc(sV[vpi % VBUFS], 16)

            def kadma(c):
                if c >= KABUFS:
                    sync.wait_ge(sDVE, DVE[f"STT0_{c - KABUFS}_{CHUNK - 1}"])
                sync.dma_start(out=ka[c % KABUFS][:],
                               in_=ka_d[:][c]).then_inc(sKA[c % KABUFS], 16)

            def ktdma(c):
                sync.dma_start(out=kt[c % KTBUFS][:],
                               in_=kt_d[:][c]).then_inc(sKT[c % KTBUFS], 16)

            # k chunks front-loaded (kt fully buffered, 4 slots); v in
            # consumption order so each period's w can start on arrival
            ktdma(1)
            vdma(0)
            kadma(1)
            vdma(1)
            kadma(2)
            ktdma(2)
            vdma(2)
            ktdma(3)
            vdma(3)
            kadma(3)
            for pi in range(4, len(PERIODS)):
                vdma(pi)

            sync.wait_ge(sDVE, DVE["PROJCP"])
            sync.dma_start(out=out_d[:], in_=o_sb[:]).then_inc(sOUT, 16)
            sync.wait_ge(sOUT, 16)

        # ---------- GPSIMD: wv on the otherwise-idle SWDGE queue ----------
        @blk.gpsimd
        def _(g_eng):
            g_eng.wait_ge(sV[0], 16)
            g_eng.dma_start(out=wv_sb[:],
                            in_=wv_d[:].rearrange("(dc p) e -> p dc e", p=128)
                            ).then_inc(sWV, 16)

        # ---------- PE ----------
        @blk.tensor
        def _(tensor):
            tensor.wait_ge(sDVE, DVE["MS2"])
            tensor.wait_ge(sW1, 32)     # q + wkq (full set)

            def qprow(b):
                # b0 -> bank 7 [256:512], b1 -> bank 4 [0:256]
                dst = ps_m[0:1, 256:256 + A] if b == 0 else ps_s[0:1, 0:A]
                for dc in range(8):
                    mm = tensor.matmul(
                        out=dst,
                        lhsT=q_col[:, dc, b:b + 1],
                        rhs=wkq_sb[:, dc, :],
                        start=(dc == 0), stop=(dc == 7),
                    )
                mm.then_inc(sPE, 1)                   # QPROW{b}

            def qpf(b):
                # bank-7 safety: the bank-7 read (QPRSB0) precedes QPF0
                tensor.wait_ge(sDVE, DVE[f"QPRSB{b}"])
                for ac in range(2):
                    mm = tensor.matmul(
                        out=ps_m[:, ac * 2 + b:ac * 2 + b + 1],
                        lhsT=qpr_sb[b][0:1, ac * 128:(ac + 1) * 128],
                        rhs=ones_bf[0:1, 0:1],
                        start=True, stop=True,
                    )
                mm.then_inc(sPE, 1)                   # QPF{b}

            def qt_mm(b):
                # b0 -> ps_w1 (banks 2-3), b1 -> ps_w0 (0-1, after warmup)
                if b == 0:
                    tensor.wait_ge(sDVE, DVE["QPSB"])
                dst = ps_w1 if b == 0 else ps_w0
                for ac in range(2):
                    for nh in range(2):
                        mm = tensor.matmul(
                            out=dst[0:1, nh * 512:(nh + 1) * 512],
                            lhsT=qp_sb[:, ac * 2 + b:ac * 2 + b + 1],
                            rhs=wkqT_sb[:, ac, nh * 512:(nh + 1) * 512],
                            start=(ac == 0), stop=(ac == 1),
                        )
                mm.then_inc(sPE, 1)                   # QT{b}

            def qtb0_mm():
                # broadcast qt0 row to 128 partitions -> banks 4-5
                tensor.wait_ge(sACT, ACT["QTSB0"])
                for nh in range(2):
                    mm = tensor.matmul(
                        out=ps_s[:, nh * 512:(nh + 1) * 512],
                        lhsT=ones_bf[:],
                        rhs=qt_sb[0][0:1, nh * 512:(nh + 1) * 512],
                        start=True, stop=True,
                    )
                mm.then_inc(sPE, 1)                   # QTB0

            def qtc1_mm():
                # qt1 row -> 8 column slices [128x1] in bank 6 (bf16)
                tensor.wait_ge(sACT, ACT["QTSB1"])
                for s in range(8):
                    mm = tensor.transpose(
                        out=ps_t[:, 2 * s:2 * s + 1],
                        in_=qt_sb[1][0:1, s * 128:(s + 1) * 128],
                        identity=ones_bf[0:1, 0:1],
                    )
                mm.then_inc(sPE, 1)                   # QTC1

            qprow(0)
            qprow(1)
            qpf(0)
            qpf(1)
            qt_mm(0)
            qt_mm(1)
            qtb0_mm()
            qtc1_mm()

            def s1_mm(c, half):
                # row-form scores: lhsT = qt col slice, rhs = kT fp8 moving
                if half == 0:
                    tensor.wait_ge(sKT[c % KTBUFS], (c // KTBUFS + 1) * 16)
                    if c == 0:
                        tensor.wait_ge(sDVE, DVE["QTCSB1"])
                        tensor.wait_ge(sDVE, DVE["QTBSB0"])  # banks 4-5 free
                if c > 0:
                    tensor.wait_ge(sACT, ACT[f"EXPR1_{c - 1}_{half}"])
                for s in range(8):
                    mm = tensor.matmul(
                        out=ps_s[0:1, half * 512:(half + 1) * 512],
                        lhsT=qtc_sb[:, 2 * s:2 * s + 1],
                        rhs=kt[c % KTBUFS][:, s, half * 512:(half + 1) * 512],
                        start=(s == 0), stop=(s == 7),
                    )
                mm.then_inc(sPE, 1)                   # S1_{c}_{half}

            def t1_mm(c, half):
                # transpose prob row chunks -> bank-6 columns (bf16)
                tensor.wait_ge(sACT, ACT[f"EXPR1_{c}_{half}"])
                if half == 0:
                    if c == 0:
                        tensor.wait_ge(sDVE, DVE["QTCSB1"])  # bank 6 free
                    else:
                        tensor.wait_ge(sACT, ACT[f"PCOL1_{c - 1}"])
                for i in range(4):
                    col = 2 * (half * 4 + i)
                    mm = tensor.transpose(
                        out=ps_t[:, col:col + 1],
                        in_=p1row[c % 2][0:1,
                                         half * 512 + i * 128:
                                         half * 512 + (i + 1) * 128],
                        identity=ones_bf[0:1, 0:1],
                    )
                mm.then_inc(sPE, 1)                   # T1_{c}_{half}

            def w_sub(b, c, h, pi):
                if h == 0:
                    tensor.wait_ge(sV[pi % VBUFS], (pi // VBUFS + 1) * 16)
                if b == 0:
                    tensor.wait_ge(sACT, ACT[f"EXP0_{c}_{h}"])
                    if (c, h) == (0, 0):
                        tensor.wait_ge(sACT, ACT["QTSB1"])   # ps_w0 freed
                else:
                    if h == 0:
                        tensor.wait_ge(sACT, ACT[f"PCOL1_{c}"])
                    if (c, h) == (0, 0):
                        tensor.wait_ge(sACT, ACT["QTSB0"])   # ps_w1 freed
                acc = ps_w0 if b == 0 else ps_w1
                for j in range(h * SUB, (h + 1) * SUB):
                    t = c * CHUNK + j
                    pcol_ap = pmat0[:, t:t + 1] if b == 0 \
                        else pmat1[:, 2 * t:2 * t + 1]
                    for nh in range(2):
                        mm = tensor.matmul(
                            out=acc[0:1, nh * 512:(nh + 1) * 512],
                            lhsT=pcol_ap,
                            rhs=vt[pi % VBUFS][:, j, nh * 512:(nh + 1) * 512],
                            start=(t == 0), stop=(t == NSUB - 1),
                        )
                mm.then_inc(sPE, 1)                   # W{b}_{c}_{h}

            def z0_mm():
                tensor.wait_ge(sDVE, DVE["ZRED0"])
                tensor.matmul(
                    out=ps_m[0:1, 300:301], lhsT=zredc0[:],
                    rhs=ones_col[:], start=True, stop=True,
                ).then_inc(sPE, 1)                    # Z0

            def fold_mm(b):
                tensor.wait_ge(sACT, ACT[f"WROW{b}"])
                for dc in range(8):
                    mm = tensor.matmul(
                        out=ps_m[:, 32 + dc * 2 + b:33 + dc * 2 + b],
                        lhsT=w_row[b][0:1, dc * 128:(dc + 1) * 128],
                        rhs=ones_bf[0:1, 0:1],
                        start=True, stop=True,
                    )
                mm.then_inc(sPE, 1)                   # FOLD{b}

            def proj_mm():
                tensor.wait_ge(sDVE, DVE["WCOL"])
                tensor.wait_ge(sWV, 16)
                for dc in range(8):
                    for nh in range(2):
                        mm = tensor.matmul(
                            out=ps_s[0:2, nh * 512:(nh + 1) * 512],
                            lhsT=wcb[:, dc * 2:(dc + 1) * 2],
                            rhs=wv_sb[:, dc, nh * 512:(nh + 1) * 512],
                            start=(dc == 0), stop=(dc == 7),
                        )
                mm.then_inc(sPE, 1)                   # PROJ

            for pi, (b, c) in enumerate(PERIODS[:5]):
                if b == 0:
                    for h in range(CHUNK // SUB):
                        w_sub(b, c, h, pi)
                else:
                    s1_mm(c, 0)
                    s1_mm(c, 1)
                    t1_mm(c, 0)
                    t1_mm(c, 1)
                    for h in range(CHUNK // SUB):
                        w_sub(b, c, h, pi)
            # tail: scores first (data arrives mid-stream), then the w's
            # in v-arrival order, Z0 chain threaded between
            s1_mm(2, 0)
            s1_mm(2, 1)
            t1_mm(2, 0)
            t1_mm(2, 1)
            s1_mm(3, 0)
            s1_mm(3, 1)
            t1_mm(3, 0)
            t1_mm(3, 1)
            for h in range(CHUNK // SUB):
                w_sub(1, 2, h, 6)
            for h in range(CHUNK // SUB):
                w_sub(0, 3, h, 5)
            z0_mm()
            for h in range(CHUNK // SUB):
                w_sub(1, 3, h, 7)
            fold_mm(0)
            fold_mm(1)
            proj_mm()

        # ---------- DVE ----------
        @blk.vector
        def _(vector):
            vector.memset(ones_col[:], 1.0).then_inc(sDVE, 1)   # MS1
            vector.memset(ones_bf[:], 1.0).then_inc(sDVE, 1)    # MS2

            vector.wait_ge(sW2, 32)     # bkq + wkqT (full set)
            for b in range(BPC):
                vector.wait_ge(sPE, PE[f"QPROW{b}"])
                src = ps_m[0:1, 256:256 + A] if b == 0 else ps_s[0:1, 0:A]
                vector.tensor_add(qpr_sb[b][:], src,
                                  bkq_row[:]).then_inc(sDVE, 1)  # QPRSB{b}
            vector.wait_ge(sPE, PE["QPF1"])
            vector.tensor_copy(out=qp_sb[:], in_=ps_m[:, 0:4]) \
                .then_inc(sDVE, 1)                               # QPSB
            vector.wait_ge(sPE, PE["QTB0"])
            vector.tensor_copy(out=qtb0_sb[:], in_=ps_s[:]) \
                .then_inc(sDVE, 1)                               # QTBSB0
            vector.wait_ge(sPE, PE["QTC1"])
            vector.tensor_copy(out=qtc_sb[:], in_=ps_t[:, 0:16]) \
                .then_inc(sDVE, 1)                               # QTCSB1

            def stts(c):
                vector.wait_ge(sKA[c % KABUFS], (c // KABUFS + 1) * 16)
                if c == 0:
                    # self-wait: DVE pipelines; reads of qtb0_sb need the
                    # QTBSB0 completion, not just program order
                    vector.wait_ge(sDVE, DVE["QTBSB0"])
                else:
                    # junk-slot WAW edge for the race detector
                    vector.wait_ge(sDVE, DVE[f"STT0_{c - 1}_{CHUNK - 1}"])
                for j in range(CHUNK):
                    t = c * CHUNK + j
                    vector.scalar_tensor_tensor(
                        out=junks[j][:],
                        in0=ka[c % KABUFS][:, j, :], scalar=1.0,
                        in1=qtb0_sb[:],
                        op0=AL.mult, op1=AL.mult,
                        accum_out=smat0[:, t:t + 1],
                    ).then_inc(sDVE, 1)              # STT0_{c}_{j}

            for b, c in PERIODS:
                if b == 0:
                    stts(c)
                    if c == NCHUNK - 1:
                        vector.wait_ge(sACT,
                                       ACT[f"EXP0_{c}_{CHUNK // SUB - 1}"])
                        vector.reduce_sum(zredc0[:], pmat0[:],
                                          axis=mybir.AxisListType.X) \
                            .then_inc(sDVE, 1)                   # ZRED0
                        vector.wait_ge(sPE, PE["Z0"])
                        vector.reciprocal(invz[0][:], ps_m[0:1, 300:301]) \
                            .then_inc(sDVE, 1)                   # INVZ0

            vector.wait_ge(sACT, ACT[f"EXPR1_{NCHUNK - 1}_1"])
            vector.reduce_sum(zsum1[:], zparts[:],
                              axis=mybir.AxisListType.X) \
                .then_inc(sDVE, 1)                               # ZRED1
            # self-wait: zsum1 RAW on DVE
            vector.wait_ge(sDVE, DVE["ZRED1"])
            vector.reciprocal(invz[1][:], zsum1[:]) \
                .then_inc(sDVE, 1)                               # INVZ1

            vector.wait_ge(sPE, PE["FOLD1"])
            vector.tensor_copy(out=wcb[:], in_=ps_m[:, 32:48]) \
                .then_inc(sDVE, 1)                               # WCOL
            vector.wait_ge(sPE, PE["PROJ"])
            vector.wait_ge(sBV, 16)
            vector.tensor_add(o_sb[:], ps_s[0:2, :], bv_row[:]) \
                .then_inc(sDVE, 1)                               # PROJCP

        # ---------- ACT ----------
        @blk.scalar
        def _(scalar):
            for b in range(BPC):
                scalar.wait_ge(sPE, PE[f"QT{b}"])
                src_ps = ps_w1 if b == 0 else ps_w0
                scale = QT_SCALE0 if b == 0 else QT_SCALE1
                scalar.mul(qt_sb[b][:], src_ps[0:1, :], scale) \
                    .then_inc(sACT, 1)                           # QTSB{b}

            def exps0(c):
                for h in range(CHUNK // SUB):
                    lo = c * CHUNK + h * SUB
                    hi = lo + SUB
                    scalar.wait_ge(sDVE, DVE[f"STT0_{c}_{h * SUB + SUB - 1}"])
                    scalar.activation(
                        out=pmat0[:, lo:hi],
                        in_=smat0[:, lo:hi], func=AF.Exp,
                    ).then_inc(sACT, 1)              # EXP0_{c}_{h}

            def expr1(c, half):
                scalar.wait_ge(sPE, PE[f"S1_{c}_{half}"])
                if c >= 2 and half == 0:
                    # p1row[c%2] reuse: prior transposes must be done
                    scalar.wait_ge(sPE, PE[f"T1_{c - 2}_1"])
                zi = 2 * c + half
                scalar.activation(
                    out=p1row[c % 2][0:1, half * 512:(half + 1) * 512],
                    in_=ps_s[0:1, half * 512:(half + 1) * 512], func=AF.Exp,
                    accum_out=zparts[0:1, zi:zi + 1],
                ).then_inc(sACT, 1)                  # EXPR1_{c}_{half}

            def wrow(b):
                scalar.wait_ge(sPE, PE[f"W{b}_{NCHUNK - 1}_{CHUNK // SUB - 1}"])
                scalar.wait_ge(sDVE, DVE[f"INVZ{b}"])
                acc = ps_w0 if b == 0 else ps_w1
                scalar.activation(
                    out=w_row[b][:], in_=acc[0:1, :], func=AF.Copy,
                    bias=0.0, scale=invz[b][0:1, 0:1],
                ).then_inc(sACT, 1)                              # WROW{b}

            def pcol(c):
                scalar.wait_ge(sPE, PE[f"T1_{c}_1"])
                scalar.activation(
                    out=pmat1[:, c * 16:(c + 1) * 16],
                    in_=ps_t[:, 0:16], func=AF.Copy,
                ).then_inc(sACT, 1)                              # PCOL1_{c}

            for b, c in PERIODS:
                if b == 0:
                    exps0(c)
                else:
                    expr1(c, 0)
                    expr1(c, 1)
                    pcol(c)
            wrow(0)
            wrow(1)

    return nc


_NC_CACHE = None


def get_nc():
    global _NC_CACHE
    if _NC_CACHE is None:
        _NC_CACHE = _build_nc()
    return _NC_CACHE


def make_in_maps(q, k, v, W_kq, b_kq, W_v, b_v):
    """Shard inputs over 8 cores (batch-parallel, weights replicated).
    Per core: batch 2i -> int8 k (x32, DVE path); batch 2i+1 -> fp8-e3m4
    k^T [e-slice, tok] (PE path); v fp8-e3m4 tile-major for both."""
    import ml_dtypes

    f16 = np.float16
    e3m4 = ml_dtypes.float8_e3m4
    q = np.asarray(q, dtype=np.float32).reshape(B, E)
    qc = q.reshape(B, 8, 128).transpose(2, 1, 0).astype(f16)  # [128, 8, B]
    k = np.asarray(k, dtype=np.float32)
    v = np.asarray(v, dtype=np.float32)

    # b0 path: tile t = c*CHUNK+j covers tokens [c*1024 + j*128 + p]
    k_t = k.reshape(B, NCHUNK, CHUNK, 128, E)       # [B, c, j, p, e]
    ka_all = np.clip(np.rint(k_t * 32.0), -127, 127).astype(np.int8)
    ka_all = np.ascontiguousarray(
        ka_all.transpose(0, 1, 3, 2, 4).reshape(B, NCHUNK, 128, CHUNK * E))
    # b1 path: kt[c, p, s, t] = k[c*1024 + t, s*128 + p], fp16
    k_T = k.reshape(B, NCHUNK, 128 * CHUNK, 8, 128)  # [B, c, t, s, p]
    kt_all = np.ascontiguousarray(
        k_T.transpose(0, 1, 4, 3, 2)).astype(f16)    # [B, c, p, s, t]
    kt_all = kt_all.reshape(B, NCHUNK, 128, CHUNK * E)
    v_t = v.reshape(B, NCHUNK, CHUNK, 128, E)
    vi = np.ascontiguousarray(
        v_t.transpose(0, 1, 3, 2, 4)).astype(e3m4)
    vi = vi.reshape(B, NCHUNK, 128, CHUNK * E)

    W_kq32 = np.asarray(W_kq, dtype=np.float32)
    W_kqb = np.ascontiguousarray(W_kq32.astype(f16))
    W_kqT = np.ascontiguousarray(W_kq32.T.astype(f16))
    b_kq = np.ascontiguousarray(np.asarray(b_kq, dtype=np.float32))
    W_vq = np.ascontiguousarray(np.asarray(W_v, dtype=np.float32).astype(f16))
    b_v = np.ascontiguousarray(
        np.broadcast_to(np.asarray(b_v, dtype=np.float32)[None, :],
                        (BPC, E)).copy())

    in_maps = []
    for i in range(NCORES):
        lo = i * BPC
        in_maps.append({
            "q": np.ascontiguousarray(qc[:, :, lo:lo + BPC]),
            "ka": ka_all[lo],
            "kt": kt_all[lo + 1],
            "v": vi[lo:lo + BPC],
            "W_kq": W_kqb,
            "W_kqT": W_kqT,
            "b_kq": b_kq,
            "W_v": W_vq,
            "b_v": b_v,
        })
    return in_maps


def kernel(q, k, v, W_kq, b_kq, W_v, b_v):
    from concourse.bass_utils import run_bass_kernel_spmd

    nc = get_nc()
    in_maps = make_in_maps(q, k, v, W_kq, b_kq, W_v, b_v)
    res = run_bass_kernel_spmd(nc, in_maps, core_ids=list(range(NCORES)))
    out = np.concatenate([res.results[i]["out"] for i in range(NCORES)], axis=0)
    return np.ascontiguousarray(out.astype(np.float32))


# revision 16
# speedup vs baseline: 1.0455x; 1.0455x over previous
"""Distributed attention kernel for Trainium2 (8 NeuronCores, SPMD).

Problem: B=16 batches of single-query attention over NK=4096 keys,
EMBED=1024, ATTN=256, shared kq projection + v projection.

Math restructuring (exact up to float reassociation):
  - scores = k @ qt + const   where qt = W_kq @ (W_kq^T q + b_kq) / 16
    (constant offset -> softmax invariant, dropped)
  - out = (attn @ v) @ W_v + b_v   (attn sums to 1)

V2 design (per core: 2 batches, 4 chunks of 1024 tokens each):
  - batch 0 scores on DVE: k int8 (x32), scalar_tensor_tensor with
    qtb broadcast (qt scale 1/(16*32)), accum -> smat cols. 8 stt/chunk.
  - batch 1 scores on PE: k^T stored fp8-e3m4 [e-slice part, tok free];
    row-form matmuls lhsT=qt col slice [128x1] bf16 (scale 1/16),
    rhs=kT moving -> PSUM [1x512] rows; ACT exp rows (accum_out = Z
    partials); PE transposes [1x128]->[128x1] -> pmat cols.
  - v fp8-e3m4 for BOTH batches' attn@v (w) matmuls (lhsT pmat bf16
    x rhs fp8 moving validated on HW, rel err ~1e-7 vs exact).
  - DMA/core ~18.4MB (vs 30.5 baseline): ka 4MB int8 + kt 4MB e3m4 +
    v 8MB e3m4 + W_v 2MB bf16 + small. One HWDGE queue ~400-430 GB/s.
  - W_v on the gpsimd/SWDGE queue (otherwise idle).

Synchronization rules (enforced by CoreSim's race detector):
  - DMA-completion sems: one per BUFFER SLOT (16 SDMA engines increment
    independently; prefix waits on a shared counter are unsound).
  - Same-engine RAW needs a SELF-WAIT on the engine's own sem (DVE
    reads of qtb0 after QTBSB0; DVE INVZ1 after ZRED1).
  - PSUM bank rule: concurrent PE-write + DVE/ACT-read of one bank is
    fatal; every bank handoff is serialized via the event chain.

PSUM bank map:
  banks 0-1: warmup -> qt1 -> w0 accumulator
  banks 2-3: qt0 -> w1 accumulator
  banks 4-5: qprow1 [0:1,0:256] (b4) -> qtb0 broadcast -> score rows
             [1x512] x2 (b4=half0, b5=half1) -> proj out [2x1024]
  bank 6   : qtcol transposes -> per-chunk prob transposes (bf16)
  bank 7   : qprow0 [0:1,256:512], qp folds [:,0:4], w folds [:,32:48],
             Z0 [0:1,300:301]

Raw bass (not Tile): walrus rejects >1 embedded sync-wait per compute
instruction; standalone sequencer waits with python-precomputed
semaphore tick tables (ticks = position in each engine's event list;
the lists must match engine program order exactly).
"""

import contextlib

import numpy as np

try:
    import concourse.bass as bass  # noqa: F401
except ImportError:
    import sys

    sys.path.insert(0, "/opt/trn_rl_repo")

B = 16
NCORES = 8
BPC = B // NCORES
NK = 4096
E = 1024
A = 256
CHUNK = 8                      # k-tiles (128 tok) per chunk
NCHUNK = NK // (128 * CHUNK)   # 4 chunks per batch
NSUB = NK // 128               # 32 tiles per batch
SUB = 2                        # b0 exp granularity in tiles
KABUFS = 3
KTBUFS = 4
VBUFS = 5
QT_SCALE0 = 1.0 / (16.0 * 32.0)   # b0: int8 k carries x32
QT_SCALE1 = 1.0 / 16.0            # b1: e3m4 k carries real values

# period order: b1 (PE-scored) first so the PE has work at startup;
# all b0 (DVE) chunks early so the stt pipeline drains before the tail;
# the last two periods are b1 -> short PE-only tail chain
PERIODS = [(1, 0), (0, 0), (0, 1), (1, 1), (0, 2), (0, 3), (1, 2), (1, 3)]


def _build_nc():
    import concourse.bass as bass
    from concourse import mybir

    FP = mybir.dt.float32
    BF = mybir.dt.bfloat16
    I8 = mybir.dt.int8
    E3 = mybir.dt.float8e3
    F16 = mybir.dt.float16
    AL = mybir.AluOpType
    AF = mybir.ActivationFunctionType

    nc = bass.Bass()
    q_d = nc.declare_dram_parameter("q", [128, 8, BPC], F16, isOutput=False)
    ka_d = nc.declare_dram_parameter("ka", [NCHUNK, 128, CHUNK * E], I8,
                                     isOutput=False)
    kt_d = nc.declare_dram_parameter("kt", [NCHUNK, 128, CHUNK * E], F16,
                                     isOutput=False)
    v_d = nc.declare_dram_parameter("v", [BPC, NCHUNK, 128, CHUNK * E], E3,
                                    isOutput=False)
    wkq_d = nc.declare_dram_parameter("W_kq", [E, A], F16, isOutput=False)
    wkqT_d = nc.declare_dram_parameter("W_kqT", [A, E], F16, isOutput=False)
    bkq_d = nc.declare_dram_parameter("b_kq", [A], FP, isOutput=False)
    wv_d = nc.declare_dram_parameter("W_v", [E, E], F16, isOutput=False)
    bv_d = nc.declare_dram_parameter("b_v", [BPC, E], FP, isOutput=False)
    out_d = nc.declare_dram_parameter("out", [BPC, E], FP, isOutput=True)

    with contextlib.ExitStack() as st:
        def sb(name, shape, dt=FP):
            return st.enter_context(nc.sbuf_tensor(name, shape, dt))

        # ---- SBUF ----
        wkq_sb = sb("wkq_sb", [128, 8, A], F16)
        wkqT_sb = sb("wkqT_sb", [128, 2, E], F16)
        wv_sb = sb("wv_sb", [128, 8, E], F16)
        q_col = sb("q_col", [128, 8, BPC], F16)
        bkq_row = sb("bkq_row", [1, A])
        bv_row = sb("bv_row", [BPC, E])
        ones_col = sb("ones_col", [128, 1])      # fp32 (Z0 rhs)
        ones_bf = sb("ones_bf", [1, 128], F16)    # bf16 (fold/bcast/identity)
        ka = [sb(f"ka{i}", [128, CHUNK, E], I8) for i in range(KABUFS)]
        kt = [sb(f"kt{i}", [128, CHUNK, E], F16) for i in range(KTBUFS)]
        vt = [sb(f"vt{i}", [128, CHUNK, E], E3) for i in range(VBUFS)]
        junks = [sb(f"junk{i}", [128, E], BF) for i in range(CHUNK)]
        qpr_sb = [sb(f"qpr_sb{b}", [1, A], F16) for b in range(BPC)]
        qp_sb = sb("qp_sb", [128, 4], F16)        # cols (ac,b): ac*2+b
        qt_sb = [sb(f"qt_sb{b}", [1, E], F16) for b in range(BPC)]
        qtb0_sb = sb("qtb0_sb", [128, E], F16)    # b0 broadcast
        qtc_sb = sb("qtc_sb", [128, 16], F16)     # b1 qt cols (stride 2)
        smat0 = sb("smat0", [128, NSUB])
        pmat0 = sb("pmat0", [128, NSUB], F16)
        pmat1 = sb("pmat1", [128, 2 * NSUB], F16)  # tile t at col 2t
        p1row = [sb(f"p1row{i}", [1, E], F16) for i in range(2)]
        zparts = sb("zparts", [1, 2 * NCHUNK])   # b1 Z partials (c,half)
        zsum1 = sb("zsum1", [1, 1])
        zredc0 = sb("zredc0", [128, 1])
        invz = [sb(f"invz{b}", [1, 1]) for b in range(BPC)]
        w_row = [sb(f"w_row{b}", [1, E], F16) for b in range(BPC)]
        wcb = sb("wcb", [128, 16], F16)           # fold cols (dc,b): dc*2+b
        o_sb = sb("o_sb", [BPC, E])

        # ---- PSUM ----
        ps_w0 = st.enter_context(nc.psum_tensor([128, 1024], FP))  # 0-1
        ps_w1 = st.enter_context(nc.psum_tensor([128, 1024], FP))  # 2-3
        ps_s = st.enter_context(nc.psum_tensor([128, 1024], FP))   # 4-5
        ps_t = st.enter_context(nc.psum_tensor([128, 512], F16))    # 6
        ps_m = st.enter_context(nc.psum_tensor([128, 512], FP))    # 7

        # ---- semaphores ----
        sW1 = st.enter_context(nc.semaphore("sW1"))  # q+wkq -> 32
        sW2 = st.enter_context(nc.semaphore("sW2"))  # bkq+wkqT -> 32
        sWV = st.enter_context(nc.semaphore("sWV"))
        sBV = st.enter_context(nc.semaphore("sBV"))
        sKA = [st.enter_context(nc.semaphore(f"sKA{i}")) for i in range(KABUFS)]
        sKT = [st.enter_context(nc.semaphore(f"sKT{i}")) for i in range(KTBUFS)]
        sV = [st.enter_context(nc.semaphore(f"sV{i}")) for i in range(VBUFS)]
        sOUT = st.enter_context(nc.semaphore("sOUT"))
        sPE = st.enter_context(nc.semaphore("sPE"))
        sDVE = st.enter_context(nc.semaphore("sDVE"))
        sACT = st.enter_context(nc.semaphore("sACT"))

        blk = st.enter_context(nc.Block())

        def ticks(seq):
            assert len(set(seq)) == len(seq), "dup event"
            return {ev: i + 1 for i, ev in enumerate(seq)}

        # ---------- event sequences (must match program order) ----------
        pe_seq = ["WARM", "QPROW0", "QPROW1", "QPF0", "QPF1", "QT0", "QT1",
                  "QTB0", "QTC1"]
        # periods p0..p4 in order; hand-scheduled tail for p5..p7 so the
        # late-arriving data (v of the last periods) doesn't block earlier
        # score work in PE program order
        for pi, (b, c) in enumerate(PERIODS[:5]):
            if b == 0:
                for h in range(CHUNK // SUB):
                    pe_seq.append(f"W0_{c}_{h}")
            else:
                pe_seq.append(f"S1_{c}_0")
                pe_seq.append(f"S1_{c}_1")
                pe_seq.append(f"T1_{c}_0")
                pe_seq.append(f"T1_{c}_1")
                for h in range(CHUNK // SUB):
                    pe_seq.append(f"W1_{c}_{h}")
        pe_seq += ["S1_2_0", "S1_2_1", "T1_2_0", "T1_2_1",
                   "S1_3_0", "S1_3_1", "T1_3_0", "T1_3_1"]
        for h in range(CHUNK // SUB):
            pe_seq.append(f"W1_2_{h}")
        for h in range(CHUNK // SUB):
            pe_seq.append(f"W0_3_{h}")
        pe_seq.append("Z0")
        for h in range(CHUNK // SUB):
            pe_seq.append(f"W1_3_{h}")
        pe_seq += ["FOLD0", "FOLD1", "PROJ"]
        PE = ticks(pe_seq)

        dve_seq = ["MS1", "MS2", "QPRSB0", "QPRSB1", "QPSB", "QTBSB0",
                   "QTCSB1"]
        for b, c in PERIODS:
            if b == 0:
                for j in range(CHUNK):
                    dve_seq.append(f"STT0_{c}_{j}")
                if c == NCHUNK - 1:
                    dve_seq += ["ZRED0", "INVZ0"]
        dve_seq += ["ZRED1", "INVZ1", "WCOL", "PROJCP"]
        DVE = ticks(dve_seq)

        act_seq = ["QTSB0", "QTSB1"]
        for b, c in PERIODS:
            if b == 0:
                for h in range(CHUNK // SUB):
                    act_seq.append(f"EXP0_{c}_{h}")
            else:
                act_seq.append(f"EXPR1_{c}_0")
                act_seq.append(f"EXPR1_{c}_1")
                act_seq.append(f"PCOL1_{c}")
        act_seq += ["WROW0", "WROW1"]
        ACT = ticks(act_seq)

        # ---------- SYNC: HWDGE DMAs ----------
        @blk.sync
        def _(sync):
            sync.dma_start(out=q_col[:], in_=q_d[:]).then_inc(sW1, 16)
            sync.dma_start(
                out=wkq_sb[:], in_=wkq_d[:].rearrange("(dc p) a -> p dc a", p=128)
            ).then_inc(sW1, 16)
            sync.dma_start(out=kt[0][:], in_=kt_d[:][0]).then_inc(sKT[0], 16)
            sync.dma_start(out=ka[0][:], in_=ka_d[:][0]).then_inc(sKA[0], 16)
            sync.dma_start(out=bkq_row[:], in_=bkq_d[:][None, :]).then_inc(sW2, 16)
            sync.dma_start(
                out=wkqT_sb[:], in_=wkqT_d[:].rearrange("(ac p) d -> p ac d", p=128)
            ).then_inc(sW2, 16)
            sync.dma_start(out=bv_row[:], in_=bv_d[:]).then_inc(sBV, 16)

            def vdma(vpi):
                vb, vc = PERIODS[vpi]
                if vpi >= VBUFS:
                    bp, cp = PERIODS[vpi - VBUFS]
                    sync.wait_ge(sPE, PE[f"W{bp}_{cp}_{CHUNK // SUB - 1}"])
                sync.dma_start(out=vt[vpi % VBUFS][:],
                               in_=v_d[:][vb, vc]).then_inc(sV[vpi % VBUFS], 16)

            def kadma(c):
                if c >= KABUFS:
                    sync.wait_ge(sDVE, DVE[f"STT0_{c - KABUFS}_{CHUNK - 1}"])
                sync.dma_start(out=ka[c % KABUFS][:],
                               in_=ka_d[:][c]).then_inc(sKA[c % KABUFS], 16)

            def ktdma(c):
                sync.dma_start(out=kt[c % KTBUFS][:],
                               in_=kt_d[:][c]).then_inc(sKT[c % KTBUFS], 16)

            # k chunks front-loaded (kt fully buffered, 4 slots); v in
            # consumption order so each period's w can start on arrival
            ktdma(1)
            vdma(0)
            kadma(1)
            vdma(1)
            kadma(2)
            ktdma(2)
            vdma(2)
            ktdma(3)
            vdma(3)
            kadma(3)
            for pi in range(4, len(PERIODS)):
                vdma(pi)

            sync.wait_ge(sDVE, DVE["PROJCP"])
            sync.dma_start(out=out_d[:], in_=o_sb[:]).then_inc(sOUT, 16)
            sync.wait_ge(sOUT, 16)

        # ---------- GPSIMD: wv on the otherwise-idle SWDGE queue ----------
        @blk.gpsimd
        def _(g_eng):
            g_eng.wait_ge(sV[0], 16)
            g_eng.dma_start(out=wv_sb[:],
                            in_=wv_d[:].rearrange("(dc p) e -> p dc e", p=128)
                            ).then_inc(sWV, 16)

        # ---------- PE ----------
        @blk.tensor
        def _(tensor):
            tensor.wait_ge(sDVE, DVE["MS2"])
            # HAM warmup: dummy MMs so the setup chain runs at full clock
            # (ps_w0 is overwritten by QT1's start=True)
            for _wu in range(26):
                mm = tensor.matmul(out=ps_w0[0:128, 0:128], lhsT=ones_bf[:],
                                   rhs=ones_bf[:], start=True, stop=True)
            mm.then_inc(sPE, 1)                       # WARM
            tensor.wait_ge(sW1, 32)     # q + wkq (full set)

            def qprow(b):
                # b0 -> bank 7 [256:512], b1 -> bank 4 [0:256]
                dst = ps_m[0:1, 256:256 + A] if b == 0 else ps_s[0:1, 0:A]
                for dc in range(8):
                    mm = tensor.matmul(
                        out=dst,
                        lhsT=q_col[:, dc, b:b + 1],
                        rhs=wkq_sb[:, dc, :],
                        start=(dc == 0), stop=(dc == 7),
                    )
                mm.then_inc(sPE, 1)                   # QPROW{b}

            def qpf(b):
                # bank-7 safety: the bank-7 read (QPRSB0) precedes QPF0
                tensor.wait_ge(sDVE, DVE[f"QPRSB{b}"])
                for ac in range(2):
                    mm = tensor.matmul(
                        out=ps_m[:, ac * 2 + b:ac * 2 + b + 1],
                        lhsT=qpr_sb[b][0:1, ac * 128:(ac + 1) * 128],
                        rhs=ones_bf[0:1, 0:1],
                        start=True, stop=True,
                    )
                mm.then_inc(sPE, 1)                   # QPF{b}

            def qt_mm(b):
                # b0 -> ps_w1 (banks 2-3), b1 -> ps_w0 (0-1, after warmup)
                if b == 0:
                    tensor.wait_ge(sDVE, DVE["QPSB"])
                dst = ps_w1 if b == 0 else ps_w0
                for ac in range(2):
                    for nh in range(2):
                        mm = tensor.matmul(
                            out=dst[0:1, nh * 512:(nh + 1) * 512],
                            lhsT=qp_sb[:, ac * 2 + b:ac * 2 + b + 1],
                            rhs=wkqT_sb[:, ac, nh * 512:(nh + 1) * 512],
                            start=(ac == 0), stop=(ac == 1),
                        )
                mm.then_inc(sPE, 1)                   # QT{b}

            def qtb0_mm():
                # broadcast qt0 row to 128 partitions -> banks 4-5
                tensor.wait_ge(sACT, ACT["QTSB0"])
                for nh in range(2):
                    mm = tensor.matmul(
                        out=ps_s[:, nh * 512:(nh + 1) * 512],
                        lhsT=ones_bf[:],
                        rhs=qt_sb[0][0:1, nh * 512:(nh + 1) * 512],
                        start=True, stop=True,
                    )
                mm.then_inc(sPE, 1)                   # QTB0

            def qtc1_mm():
                # qt1 row -> 8 column slices [128x1] in bank 6 (bf16)
                tensor.wait_ge(sACT, ACT["QTSB1"])
                for s in range(8):
                    mm = tensor.transpose(
                        out=ps_t[:, 2 * s:2 * s + 1],
                        in_=qt_sb[1][0:1, s * 128:(s + 1) * 128],
                        identity=ones_bf[0:1, 0:1],
                    )
                mm.then_inc(sPE, 1)                   # QTC1

            qprow(0)
            qprow(1)
            qpf(0)
            qpf(1)
            qt_mm(0)
            qt_mm(1)
            qtb0_mm()
            qtc1_mm()

            def s1_mm(c, half):
                # row-form scores: lhsT = qt col slice, rhs = kT fp8 moving
                if half == 0:
                    tensor.wait_ge(sKT[c % KTBUFS], (c // KTBUFS + 1) * 16)
                    if c == 0:
                        tensor.wait_ge(sDVE, DVE["QTCSB1"])
                        tensor.wait_ge(sDVE, DVE["QTBSB0"])  # banks 4-5 free
                if c > 0:
                    tensor.wait_ge(sACT, ACT[f"EXPR1_{c - 1}_{half}"])
                for s in range(8):
                    mm = tensor.matmul(
                        out=ps_s[0:1, half * 512:(half + 1) * 512],
                        lhsT=qtc_sb[:, 2 * s:2 * s + 1],
                        rhs=kt[c % KTBUFS][:, s, half * 512:(half + 1) * 512],
                        start=(s == 0), stop=(s == 7),
                    )
                mm.then_inc(sPE, 1)                   # S1_{c}_{half}

            def t1_mm(c, half):
                # transpose prob row chunks -> bank-6 columns (bf16)
                tensor.wait_ge(sACT, ACT[f"EXPR1_{c}_{half}"])
                if half == 0:
                    if c == 0:
                        tensor.wait_ge(sDVE, DVE["QTCSB1"])  # bank 6 free
                    else:
                        tensor.wait_ge(sACT, ACT[f"PCOL1_{c - 1}"])
                for i in range(4):
                    col = 2 * (half * 4 + i)
                    mm = tensor.transpose(
                        out=ps_t[:, col:col + 1],
                        in_=p1row[c % 2][0:1,
                                         half * 512 + i * 128:
                                         half * 512 + (i + 1) * 128],
                        identity=ones_bf[0:1, 0:1],
                    )
                mm.then_inc(sPE, 1)                   # T1_{c}_{half}

            def w_sub(b, c, h, pi):
                if h == 0:
                    tensor.wait_ge(sV[pi % VBUFS], (pi // VBUFS + 1) * 16)
                if b == 0:
                    tensor.wait_ge(sACT, ACT[f"EXP0_{c}_{h}"])
                    if (c, h) == (0, 0):
                        tensor.wait_ge(sACT, ACT["QTSB1"])   # ps_w0 freed
                else:
                    if h == 0:
                        tensor.wait_ge(sACT, ACT[f"PCOL1_{c}"])
                    if (c, h) == (0, 0):
                        tensor.wait_ge(sACT, ACT["QTSB0"])   # ps_w1 freed
                acc = ps_w0 if b == 0 else ps_w1
                for j in range(h * SUB, (h + 1) * SUB):
                    t = c * CHUNK + j
                    pcol_ap = pmat0[:, t:t + 1] if b == 0 \
                        else pmat1[:, 2 * t:2 * t + 1]
                    for nh in range(2):
                        mm = tensor.matmul(
                            out=acc[0:1, nh * 512:(nh + 1) * 512],
                            lhsT=pcol_ap,
                            rhs=vt[pi % VBUFS][:, j, nh * 512:(nh + 1) * 512],
                            start=(t == 0), stop=(t == NSUB - 1),
                        )
                mm.then_inc(sPE, 1)                   # W{b}_{c}_{h}

            def z0_mm():
                tensor.wait_ge(sDVE, DVE["ZRED0"])
                tensor.matmul(
                    out=ps_m[0:1, 300:301], lhsT=zredc0[:],
                    rhs=ones_col[:], start=True, stop=True,
                ).then_inc(sPE, 1)                    # Z0

            def fold_mm(b):
                tensor.wait_ge(sACT, ACT[f"WROW{b}"])
                for dc in range(8):
                    mm = tensor.matmul(
                        out=ps_m[:, 32 + dc * 2 + b:33 + dc * 2 + b],
                        lhsT=w_row[b][0:1, dc * 128:(dc + 1) * 128],
                        rhs=ones_bf[0:1, 0:1],
                        start=True, stop=True,
                    )
                mm.then_inc(sPE, 1)                   # FOLD{b}

            def proj_mm():
                tensor.wait_ge(sDVE, DVE["WCOL"])
                tensor.wait_ge(sWV, 16)
                for dc in range(8):
                    for nh in range(2):
                        mm = tensor.matmul(
                            out=ps_s[0:2, nh * 512:(nh + 1) * 512],
                            lhsT=wcb[:, dc * 2:(dc + 1) * 2],
                            rhs=wv_sb[:, dc, nh * 512:(nh + 1) * 512],
                            start=(dc == 0), stop=(dc == 7),
                        )
                mm.then_inc(sPE, 1)                   # PROJ

            for pi, (b, c) in enumerate(PERIODS[:5]):
                if b == 0:
                    for h in range(CHUNK // SUB):
                        w_sub(b, c, h, pi)
                else:
                    s1_mm(c, 0)
                    s1_mm(c, 1)
                    t1_mm(c, 0)
                    t1_mm(c, 1)
                    for h in range(CHUNK // SUB):
                        w_sub(b, c, h, pi)
            # tail: scores first (data arrives mid-stream), then the w's
            # in v-arrival order, Z0 chain threaded between
            s1_mm(2, 0)
            s1_mm(2, 1)
            t1_mm(2, 0)
            t1_mm(2, 1)
            s1_mm(3, 0)
            s1_mm(3, 1)
            t1_mm(3, 0)
            t1_mm(3, 1)
            for h in range(CHUNK // SUB):
                w_sub(1, 2, h, 6)
            for h in range(CHUNK // SUB):
                w_sub(0, 3, h, 5)
            z0_mm()
            for h in range(CHUNK // SUB):
                w_sub(1, 3, h, 7)
            fold_mm(0)
            fold_mm(1)
            proj_mm()

        # ---------- DVE ----------
        @blk.vector
        def _(vector):
            vector.memset(ones_col[:], 1.0).then_inc(sDVE, 1)   # MS1
            vector.memset(ones_bf[:], 1.0).then_inc(sDVE, 1)    # MS2

            vector.wait_ge(sW2, 32)     # bkq + wkqT (full set)
            for b in range(BPC):
                vector.wait_ge(sPE, PE[f"QPROW{b}"])
                src = ps_m[0:1, 256:256 + A] if b == 0 else ps_s[0:1, 0:A]
                vector.tensor_add(qpr_sb[b][:], src,
                                  bkq_row[:]).then_inc(sDVE, 1)  # QPRSB{b}
            vector.wait_ge(sPE, PE["QPF1"])
            vector.tensor_copy(out=qp_sb[:], in_=ps_m[:, 0:4]) \
                .then_inc(sDVE, 1)                               # QPSB
            vector.wait_ge(sPE, PE["QTB0"])
            vector.tensor_copy(out=qtb0_sb[:], in_=ps_s[:]) \
                .then_inc(sDVE, 1)                               # QTBSB0
            vector.wait_ge(sPE, PE["QTC1"])
            vector.tensor_copy(out=qtc_sb[:], in_=ps_t[:, 0:16]) \
                .then_inc(sDVE, 1)                               # QTCSB1

            def stts(c):
                vector.wait_ge(sKA[c % KABUFS], (c // KABUFS + 1) * 16)
                if c == 0:
                    # self-wait: DVE pipelines; reads of qtb0_sb need the
                    # QTBSB0 completion, not just program order
                    vector.wait_ge(sDVE, DVE["QTBSB0"])
                else:
                    # junk-slot WAW edge for the race detector
                    vector.wait_ge(sDVE, DVE[f"STT0_{c - 1}_{CHUNK - 1}"])
                for j in range(CHUNK):
                    t = c * CHUNK + j
                    vector.scalar_tensor_tensor(
                        out=junks[j][:],
                        in0=ka[c % KABUFS][:, j, :], scalar=1.0,
                        in1=qtb0_sb[:],
                        op0=AL.mult, op1=AL.mult,
                        accum_out=smat0[:, t:t + 1],
                    ).then_inc(sDVE, 1)              # STT0_{c}_{j}

            for b, c in PERIODS:
                if b == 0:
                    stts(c)
                    if c == NCHUNK - 1:
                        vector.wait_ge(sACT,
                                       ACT[f"EXP0_{c}_{CHUNK // SUB - 1}"])
                        vector.reduce_sum(zredc0[:], pmat0[:],
                                          axis=mybir.AxisListType.X) \
                            .then_inc(sDVE, 1)                   # ZRED0
                        vector.wait_ge(sPE, PE["Z0"])
                        vector.reciprocal(invz[0][:], ps_m[0:1, 300:301]) \
                            .then_inc(sDVE, 1)                   # INVZ0

            vector.wait_ge(sACT, ACT[f"EXPR1_{NCHUNK - 1}_1"])
            vector.reduce_sum(zsum1[:], zparts[:],
                              axis=mybir.AxisListType.X) \
                .then_inc(sDVE, 1)                               # ZRED1
            # self-wait: zsum1 RAW on DVE
            vector.wait_ge(sDVE, DVE["ZRED1"])
            vector.reciprocal(invz[1][:], zsum1[:]) \
                .then_inc(sDVE, 1)                               # INVZ1

            vector.wait_ge(sPE, PE["FOLD1"])
            vector.tensor_copy(out=wcb[:], in_=ps_m[:, 32:48]) \
                .then_inc(sDVE, 1)                               # WCOL
            vector.wait_ge(sPE, PE["PROJ"])
            vector.wait_ge(sBV, 16)
            vector.tensor_add(o_sb[:], ps_s[0:2, :], bv_row[:]) \
                .then_inc(sDVE, 1)                               # PROJCP

        # ---------- ACT ----------
        @blk.scalar
        def _(scalar):
            for b in range(BPC):
                scalar.wait_ge(sPE, PE[f"QT{b}"])
                src_ps = ps_w1 if b == 0 else ps_w0
                scale = QT_SCALE0 if b == 0 else QT_SCALE1
                scalar.mul(qt_sb[b][:], src_ps[0:1, :], scale) \
                    .then_inc(sACT, 1)                           # QTSB{b}

            def exps0(c):
                for h in range(CHUNK // SUB):
                    lo = c * CHUNK + h * SUB
                    hi = lo + SUB
                    scalar.wait_ge(sDVE, DVE[f"STT0_{c}_{h * SUB + SUB - 1}"])
                    scalar.activation(
                        out=pmat0[:, lo:hi],
                        in_=smat0[:, lo:hi], func=AF.Exp,
                    ).then_inc(sACT, 1)              # EXP0_{c}_{h}

            def expr1(c, half):
                scalar.wait_ge(sPE, PE[f"S1_{c}_{half}"])
                if c >= 2 and half == 0:
                    # p1row[c%2] reuse: prior transposes must be done
                    scalar.wait_ge(sPE, PE[f"T1_{c - 2}_1"])
                zi = 2 * c + half
                scalar.activation(
                    out=p1row[c % 2][0:1, half * 512:(half + 1) * 512],
                    in_=ps_s[0:1, half * 512:(half + 1) * 512], func=AF.Exp,
                    accum_out=zparts[0:1, zi:zi + 1],
                ).then_inc(sACT, 1)                  # EXPR1_{c}_{half}

            def wrow(b):
                scalar.wait_ge(sPE, PE[f"W{b}_{NCHUNK - 1}_{CHUNK // SUB - 1}"])
                scalar.wait_ge(sDVE, DVE[f"INVZ{b}"])
                acc = ps_w0 if b == 0 else ps_w1
                scalar.activation(
                    out=w_row[b][:], in_=acc[0:1, :], func=AF.Copy,
                    bias=0.0, scale=invz[b][0:1, 0:1],
                ).then_inc(sACT, 1)                              # WROW{b}

            def pcol(c):
                scalar.wait_ge(sPE, PE[f"T1_{c}_1"])
                scalar.activation(
                    out=pmat1[:, c * 16:(c + 1) * 16],
                    in_=ps_t[:, 0:16], func=AF.Copy,
                ).then_inc(sACT, 1)                              # PCOL1_{c}

            for b, c in PERIODS:
                if b == 0:
                    exps0(c)
                else:
                    expr1(c, 0)
                    expr1(c, 1)
                    pcol(c)
            wrow(0)
            wrow(1)

    return nc


_NC_CACHE = None


def get_nc():
    global _NC_CACHE
    if _NC_CACHE is None:
        _NC_CACHE = _build_nc()
    return _NC_CACHE


def make_in_maps(q, k, v, W_kq, b_kq, W_v, b_v):
    """Shard inputs over 8 cores (batch-parallel, weights replicated).
    Per core: batch 2i -> int8 k (x32, DVE path); batch 2i+1 -> fp8-e3m4
    k^T [e-slice, tok] (PE path); v fp8-e3m4 tile-major for both."""
    import ml_dtypes

    f16 = np.float16
    e3m4 = ml_dtypes.float8_e3m4
    q = np.asarray(q, dtype=np.float32).reshape(B, E)
    qc = q.reshape(B, 8, 128).transpose(2, 1, 0).astype(f16)  # [128, 8, B]
    k = np.asarray(k, dtype=np.float32)
    v = np.asarray(v, dtype=np.float32)

    # b0 path: tile t = c*CHUNK+j covers tokens [c*1024 + j*128 + p]
    k_t = k.reshape(B, NCHUNK, CHUNK, 128, E)       # [B, c, j, p, e]
    ka_all = np.clip(np.rint(k_t * 32.0), -127, 127).astype(np.int8)
    ka_all = np.ascontiguousarray(
        ka_all.transpose(0, 1, 3, 2, 4).reshape(B, NCHUNK, 128, CHUNK * E))
    # b1 path: kt[c, p, s, t] = k[c*1024 + t, s*128 + p], fp16
    k_T = k.reshape(B, NCHUNK, 128 * CHUNK, 8, 128)  # [B, c, t, s, p]
    kt_all = np.ascontiguousarray(
        k_T.transpose(0, 1, 4, 3, 2)).astype(f16)    # [B, c, p, s, t]
    kt_all = kt_all.reshape(B, NCHUNK, 128, CHUNK * E)
    v_t = v.reshape(B, NCHUNK, CHUNK, 128, E)
    vi = np.ascontiguousarray(
        v_t.transpose(0, 1, 3, 2, 4)).astype(e3m4)
    vi = vi.reshape(B, NCHUNK, 128, CHUNK * E)

    W_kq32 = np.asarray(W_kq, dtype=np.float32)
    W_kqb = np.ascontiguousarray(W_kq32.astype(f16))
    W_kqT = np.ascontiguousarray(W_kq32.T.astype(f16))
    b_kq = np.ascontiguousarray(np.asarray(b_kq, dtype=np.float32))
    W_vq = np.ascontiguousarray(np.asarray(W_v, dtype=np.float32).astype(f16))
    b_v = np.ascontiguousarray(
        np.broadcast_to(np.asarray(b_v, dtype=np.float32)[None, :],
                        (BPC, E)).copy())

    in_maps = []
    for i in range(NCORES):
        lo = i * BPC
        in_maps.append({
            "q": np.ascontiguousarray(qc[:, :, lo:lo + BPC]),
            "ka": ka_all[lo],
            "kt": kt_all[lo + 1],
            "v": vi[lo:lo + BPC],
            "W_kq": W_kqb,
            "W_kqT": W_kqT,
            "b_kq": b_kq,
            "W_v": W_vq,
            "b_v": b_v,
        })
    return in_maps


def kernel(q, k, v, W_kq, b_kq, W_v, b_v):
    from concourse.bass_utils import run_bass_kernel_spmd

    nc = get_nc()
    in_maps = make_in_maps(q, k, v, W_kq, b_kq, W_v, b_v)
    res = run_bass_kernel_spmd(nc, in_maps, core_ids=list(range(NCORES)))
    out = np.concatenate([res.results[i]["out"] for i in range(NCORES)], axis=0)
    return np.ascontiguousarray(out.astype(np.float32))
